# revision 2
# baseline (speedup 1.0000x reference)
"""Trainium2 Bass kernel v2: single-layer causal attention (q/k/v/o + RoPE).

Sharding: 8 cores = 2 batches x 4 head-groups (4 heads each), bf16 compute.
Per core inputs (all cheap host views, fp32 except tables):
  xs   [512, 1024]  distinct row-slice of x.reshape(4096, 1024)
  wq/wk/wv/wo_r [256, 1024] natural row-slices of the weight matrices
  cs/sn [32, 2048]  bf16 rope tables (row = freq, col = position)
Device: transpose x-slice on PE -> bf16, AllGather x^T within batch group;
transpose weights on PE (q/k rows permuted to [h0e h0o h1e h1o] pair tiles);
projections, swap-DMA RoPE, attention with per-head 64-contraction matmuls
(2 heads concurrent via tile_position), ones-column rowsums, ctx AllGather,
column-sharded o_proj (no host reduction). Output [2048, 256] per core.
"""

import os
import sys

import numpy as np

sys.path.insert(0, "/opt/trn_rl_repo")

import concourse.bass as bass  # noqa: E402
import concourse.tile as tile  # noqa: E402
from concourse import bacc, mybir  # noqa: E402
from concourse import bass_utils  # noqa: E402

B, S, D, H, DK = 2, 2048, 1024, 16, 64
NCORES = 8
HPC = H // 4        # 4 heads per core
CW = HPC * DK       # 256 head-dim columns per core
VW = DK + 1         # 65: v width per head incl ones column
ND = D // 128       # 8 contraction chunks
NS = S // 128       # 16 s-tiles
NSC = S // 512      # 4 s-chunks
ROPE_THETA = 10000.0

F32 = mybir.dt.float32
BF16 = mybir.dt.bfloat16
I32 = mybir.dt.int32
EXP = mybir.ActivationFunctionType.Exp
GROUPS = [[0, 1, 2, 3], [4, 5, 6, 7]]


def _build_kernel(tc, nc, xs, wq, wk, wv, wo, cs, sn, out, ident):
    from contextlib import ExitStack
    st_ = ExitStack()
    constp = st_.enter_context(tc.tile_pool(name="const", bufs=1))
    pers = st_.enter_context(tc.tile_pool(name="persist", bufs=1))
    dramp = st_.enter_context(tc.tile_pool(name="dram", bufs=1, space="DRAM"))

    id_sb = constp.tile([128, 128], F32)
    nc.sync.dma_start(id_sb[:], ident[:])

    # persistent SBUF tensors
    xts = pers.tile([128, ND * S], BF16)          # x^T, d-chunk major
    q_sb = [pers.tile([128, S], BF16, name=f"q_sb{_i}") for _i in range(2)]
    k_sb = [pers.tile([128, S], BF16, name=f"k_sb{_i}") for _i in range(2)]
    v_sb = pers.tile([128, NS * HPC * VW], BF16)
    wq_sb = pers.tile([128, ND * CW], BF16)
    wk_sb = pers.tile([128, ND * CW], BF16)
    wv_sb = pers.tile([128, ND * CW], BF16)
    wo_sb = pers.tile([128, ND * CW], BF16)
    cs_sb = pers.tile([128, S], BF16)             # cos, all 4 row-blocks
    sn_sb = pers.tile([128, S], BF16)             # [-sin, +sin, -sin, +sin]
    mkm = pers.tile([128, 4 * 512], BF16)         # causal masks j=0..3
    ctxg = pers.tile([128, ND * 512], BF16)       # gathered ctx^T per chunk

    # DRAM scratch for collectives
    xtp_d = dramp.tile([D, 512], BF16)
    xg_d = dramp.tile([4 * D, 512], BF16)
    ctxd = [dramp.tile([2 * 128, 512], BF16, name=f"ctxd{_i}") for _i in range(NSC)]
    ctxg_d = [dramp.tile([D, 512], BF16, name=f"ctxg_d{_i}") for _i in range(NSC)]

    v3 = v_sb[:].rearrange("p (s h c) -> p s h c", h=HPC, c=VW)

    # ---- Phase 0a: x slice -> transpose -> DRAM -> AllGather ----
    with tc.tile_pool(name="xstg", bufs=2) as xstg, \
         tc.tile_pool(name="tps", bufs=4, space="PSUM") as tps, \
         tc.tile_pool(name="xbp", bufs=4) as xbp:
        for sb in range(4):
            xt_in = xstg.tile([128, D], F32, tag="xin")
            nc.sync.dma_start(xt_in[:], xs[sb * 128:(sb + 1) * 128, :])
            for dc in range(ND):
                pt = tps.tile([128, 128], F32)
                nc.tensor.transpose(pt[:], xt_in[:, dc * 128:(dc + 1) * 128],
                                    id_sb[:])
                tb = xbp.tile([128, 128], BF16)
                if dc % 2 == 0:
                    nc.scalar.copy(tb[:], pt[:])
                else:
                    nc.vector.tensor_copy(tb[:], pt[:])
                nc.sync.dma_start(
                    xtp_d[dc * 128:(dc + 1) * 128,
                          sb * 128:(sb + 1) * 128], tb[:])
        nc.gpsimd.collective_compute(
            "AllGather", mybir.AluOpType.bypass, replica_groups=GROUPS,
            ins=[xtp_d.opt()], outs=[xg_d.opt()])

        # ---- Phase 0b: tables, masks, ones (overlap the AllGather) ----
        for r in range(4):
            nc.sync.dma_start(cs_sb[r * 32:(r + 1) * 32, :], cs[:])
            nc.sync.dma_start(sn_sb[r * 32:(r + 1) * 32, :], sn[:])
        # negate sin rows for the "even" row blocks (rows 0:32 and 64:96)
        nc.vector.tensor_scalar_mul(sn_sb[0:32, :], sn_sb[0:32, :], -1.0)
        nc.vector.tensor_scalar_mul(sn_sb[64:96, :], sn_sb[64:96, :], -1.0)
        it = xstg.tile([128, 512], I32)
        nc.gpsimd.iota(it[:], pattern=[[1, 512]], base=0, channel_multiplier=-1)
        for j in range(4):
            nc.vector.tensor_scalar(mkm[:, j * 512:(j + 1) * 512], it[:],
                                    128 * j, None, mybir.AluOpType.is_ge)
        nc.vector.memset(v3[:, :, :, DK:DK + 1], 1.0)

        # ---- Phase 0c: weight loads (permuted for q/k) + transposes ----
        for w_dram, w_sb, permute in ((wq, wq_sb, True), (wk, wk_sb, True),
                                      (wv, wv_sb, False), (wo, wo_sb, False)):
            for mt in range(2):  # head-pair (q/k) or m-tile (v/o)
                stg = xstg.tile([128, D], F32, tag="wstg")
                if permute:
                    # rows [h0e h0o h1e h1o] for pair mt
                    for hh in range(2):
                        base = mt * 128 + hh * 64
                        for par in range(2):
                            nc.sync.dma_start(
                                stg[hh * 64 + par * 32:hh * 64 + par * 32 + 32, :],
                                w_dram[base + par:base + 64:2, :])
                else:
                    nc.sync.dma_start(stg[:], w_dram[mt * 128:(mt + 1) * 128, :])
                for dc in range(ND):
                    pt = tps.tile([128, 128], F32)
                    nc.tensor.transpose(pt[:], stg[:, dc * 128:(dc + 1) * 128],
                                        id_sb[:])
                    dst = w_sb[:, dc * 256 + mt * 128:dc * 256 + mt * 128 + 128]
                    if dc % 2 == 0:
                        nc.scalar.copy(dst, pt[:])
                    else:
                        nc.vector.tensor_copy(dst, pt[:])

        # ---- Phase 0d: load gathered x^T into SBUF ----
        for dc in range(ND):
            for r in range(4):
                nc.sync.dma_start(
                    xts[:, dc * S + r * 512:dc * S + (r + 1) * 512],
                    xg_d[r * D + dc * 128:r * D + (dc + 1) * 128, :])

    # ---- Phases 1-3 interleaved over s-chunks ----
    with tc.tile_pool(name="pjps", bufs=2, space="PSUM") as pjps, \
         tc.tile_pool(name="vps", bufs=1, space="PSUM") as vps, \
         tc.tile_pool(name="sps", bufs=2, space="PSUM") as sps, \
         tc.tile_pool(name="cps", bufs=2, space="PSUM") as cps, \
         tc.tile_pool(name="ops", bufs=1, space="PSUM") as opsp, \
         tc.tile_pool(name="expool", bufs=8) as exp_pool, \
         tc.tile_pool(name="smp", bufs=4) as smp, \
         tc.tile_pool(name="swp", bufs=2) as swp, \
         tc.tile_pool(name="ctxp", bufs=2) as ctxp, \
         tc.tile_pool(name="obuf", bufs=4) as obp:

        def project(sc):
            for hp in range(2):
                for dst_sb, wsb in ((q_sb[hp], wq_sb), (k_sb[hp], wk_sb)):
                    ps = pjps.tile([128, 512], F32, name="ps")
                    for dc in range(ND):
                        nc.tensor.matmul(
                            ps[:],
                            wsb[:, dc * 256 + hp * 128:dc * 256 + hp * 128 + 128],
                            xts[:, dc * S + sc * 512:dc * S + (sc + 1) * 512],
                            start=(dc == 0), stop=(dc == ND - 1))
                    nc.vector.tensor_copy(
                        dst_sb[:, sc * 512:(sc + 1) * 512], ps[:])
            for st4 in range(4):
                stt = sc * 4 + st4
                pv = vps.tile([128, 256], F32, name="pv")
                for dc in range(ND):
                    nc.tensor.matmul(
                        pv[:],
                        xts[:, dc * S + stt * 128:dc * S + stt * 128 + 128],
                        wv_sb[:, dc * 256:(dc + 1) * 256],
                        start=(dc == 0), stop=(dc == ND - 1))
                nc.scalar.copy(v3[:, stt, :, 0:DK],
                               pv[:].rearrange("p (h c) -> p h c", c=DK))

        def rope(sc):
            sl = slice(sc * 512, (sc + 1) * 512)
            for t_sb in (q_sb[0], k_sb[0], q_sb[1], k_sb[1]):
                sw = swp.tile([128, 512], BF16, tag="sw")
                tm = swp.tile([128, 512], BF16, tag="tm")
                for h2 in range(2):
                    r = h2 * 64
                    nc.sync.dma_start(sw[r:r + 32, :], t_sb[r + 32:r + 64, sl])
                    nc.sync.dma_start(sw[r + 32:r + 64, :], t_sb[r:r + 32, sl])
                nc.vector.tensor_mul(tm[:], sw[:], sn_sb[:, sl])
                nc.vector.tensor_mul(t_sb[:, sl], t_sb[:, sl], cs_sb[:, sl])
                nc.vector.tensor_add(t_sb[:, sl], t_sb[:, sl], tm[:])

        def attention(c):
            nsk = 4 * (c + 1)
            qsl = slice(c * 512, (c + 1) * 512)
            for hp in range(2):
                pctx = [cps.tile([VW, 512], F32, name=f"pc{_h}", tag="pc")
                        for _h in range(2)]
                exps = [[], []]
                DEPTH = 2

                def pv_mm(t, h2, nsk=nsk, pctx=pctx, exps=exps):
                    nc.tensor.matmul(
                        pctx[h2][:],
                        v3[:, t, hp * 2 + h2, :],
                        exps[h2][t][:],
                        start=(t == 0), stop=(t == nsk - 1),
                        skip_group_check=True)

                for t in range(nsk):
                    j = t - 4 * c
                    for h2 in range(2):
                        pscore = sps.tile([128, 512], F32, name="pscore")
                        nc.tensor.matmul(
                            pscore[:],
                            k_sb[hp][h2 * 64:h2 * 64 + 64, t * 128:(t + 1) * 128],
                            q_sb[hp][h2 * 64:h2 * 64 + 64, qsl],
                            start=True, stop=True,
                            tile_position=(h2 * 64, 0),
                            skip_group_check=True)
                        et = exp_pool.tile([128, 512], BF16)
                        nc.scalar.activation(et[:], pscore[:], EXP, scale=0.125)
                        if j >= 0:
                            nc.vector.tensor_mul(et[:], et[:],
                                                 mkm[:, j * 512:(j + 1) * 512])
                        exps[h2].append(et)
                    if t >= DEPTH:
                        pv_mm(t - DEPTH, 0)
                        pv_mm(t - DEPTH, 1)
                for t in range(max(0, nsk - DEPTH), nsk):
                    pv_mm(t, 0)
                    pv_mm(t, 1)

                ctx_sb = ctxp.tile([128, 512], BF16)
                for h2 in range(2):
                    dn = smp.tile([1, 512], F32, tag="dn")
                    nc.scalar.copy(dn[:], pctx[h2][DK:DK + 1, :])
                    rb = smp.tile([64, 512], F32, tag="rb")
                    nc.gpsimd.partition_broadcast(rb[:], dn[:])
                    nc.vector.reciprocal(rb[:], rb[:])
                    nc.vector.tensor_mul(ctx_sb[h2 * 64:h2 * 64 + 64, :],
                                         pctx[h2][0:DK, :], rb[:])
                nc.sync.dma_start(ctxd[c][hp * 128:(hp + 1) * 128, :],
                                  ctx_sb[:])
            nc.gpsimd.collective_compute(
                "AllGather", mybir.AluOpType.bypass, replica_groups=GROUPS,
                ins=[ctxd[c].opt()], outs=[ctxg_d[c].opt()])

        def o_proj(c):
            for dc in range(ND):
                nc.sync.dma_start(ctxg[:, dc * 512:(dc + 1) * 512],
                                  ctxg_d[c][dc * 128:(dc + 1) * 128, :])
            for st4 in range(4):
                pso = opsp.tile([128, 256], F32, name="pso")
                for dc in range(ND):
                    nc.tensor.matmul(
                        pso[:],
                        ctxg[:, dc * 512 + st4 * 128:dc * 512 + st4 * 128 + 128],
                        wo_sb[:, dc * 256:(dc + 1) * 256],
                        start=(dc == 0), stop=(dc == ND - 1))
                ot = obp.tile([128, 256], F32)
                nc.vector.tensor_copy(ot[:], pso[:])
                nc.sync.dma_start(
                    out[c * 512 + st4 * 128:c * 512 + st4 * 128 + 128, :],
                    ot[:])

        # Pipeline order keeps the PE dense: projections run ahead, rope of
        # chunk k overlaps attention of chunk k-1, o_proj trails by one chunk
        # so its AllGather latency is hidden.
        project(0)
        rope(0)
        project(1)
        attention(0)
        rope(1)
        project(2)
        attention(1)
        rope(2)
        o_proj(0)
        project(3)
        attention(2)
        rope(3)
        o_proj(1)
        attention(3)
        o_proj(2)
        o_proj(3)

    st_.close()


def build_nc():
    nc = bacc.Bacc("TRN2", target_bir_lowering=False, debug=False,
                   enable_asserts=False, num_devices=NCORES)
    xs = nc.dram_tensor("xs", [512, D], F32, kind="ExternalInput").ap()
    wq = nc.dram_tensor("wq", [CW, D], F32, kind="ExternalInput").ap()
    wk = nc.dram_tensor("wk", [CW, D], F32, kind="ExternalInput").ap()
    wv = nc.dram_tensor("wv", [CW, D], F32, kind="ExternalInput").ap()
    wo = nc.dram_tensor("wo", [CW, D], F32, kind="ExternalInput").ap()
    cs = nc.dram_tensor("cs", [32, S], BF16, kind="ExternalInput").ap()
    sn = nc.dram_tensor("sn", [32, S], BF16, kind="ExternalInput").ap()
    out = nc.dram_tensor("out_part", [S, CW], F32, kind="ExternalOutput").ap()
    ident = nc.inline_tensor(np.eye(128, dtype=np.float32), name="ident").ap()
    with tile.TileContext(nc) as tc:
        _build_kernel(tc, nc, xs, wq, wk, wv, wo, cs, sn, out, ident)
    nc.compile()
    return nc


def _tables():
    import ml_dtypes
    inv = ROPE_THETA ** (-2.0 * np.arange(DK // 2, dtype=np.float64) / DK)
    ang = inv[:, None] * np.arange(S, dtype=np.float64)[None, :]  # [32, S]
    cs = np.cos(ang).astype(ml_dtypes.bfloat16)
    sn = np.sin(ang).astype(ml_dtypes.bfloat16)
    return cs, sn


def make_in_maps(in_features, q_proj_weight, k_proj_weight, v_proj_weight,
                 o_proj_weight, token_positions):
    x = np.ascontiguousarray(np.asarray(in_features, dtype=np.float32))
    ws = [np.ascontiguousarray(np.asarray(w, dtype=np.float32))
          for w in (q_proj_weight, k_proj_weight, v_proj_weight, o_proj_weight)]
    pos = np.asarray(token_positions)
    assert np.array_equal(pos, np.arange(S)), "kernel assumes arange positions"
    cs, sn = _tables()
    xf = x.reshape(NCORES * 512, D)
    in_maps = []
    for c in range(NCORES):
        g = c % 4
        m = {"xs": xf[c * 512:(c + 1) * 512]}
        for nmm, w in zip(("wq", "wk", "wv", "wo"), ws):
            m[nmm] = w[g * CW:(g + 1) * CW]
        m["cs"] = cs
        m["sn"] = sn
        in_maps.append(m)
    return in_maps


# ---------------- cached PJRT runner (axon path) ----------------
_RUN_CACHE = {}
last_exec_ns = None


def _axon_active():
    try:
        from concourse._compat import axon_active
        return axon_active()
    except Exception:
        return False


def _make_cached_runner(nc, n_cores=NCORES):
    import jax
    from jax.sharding import Mesh, PartitionSpec
    from jax.experimental.shard_map import shard_map
    from concourse import bass2jax

    bass2jax.install_neuronx_cc_hook()
    in_names, out_names, out_avals = [], [], []
    partition_name = nc.partition_id_tensor.name if nc.partition_id_tensor else None
    for alloc in nc.m.functions[0].allocations:
        if not isinstance(alloc, mybir.MemoryLocationSet):
            continue
        if alloc.kind not in ("ExternalInput", "ExternalOutput"):
            continue
        name = alloc.memorylocations[0].name
        if alloc.kind == "ExternalInput":
            if name != partition_name:
                in_names.append(name)
        else:
            out_names.append(name)
            out_avals.append(jax.core.ShapedArray(tuple(alloc.tensor_shape),
                                                  mybir.dt.np(alloc.dtype)))
    n_params = len(in_names)
    all_in_names = list(in_names) + list(out_names)
    if partition_name is not None:
        all_in_names.append(partition_name)

    def _body(*args):
        operands = list(args)
        if partition_name is not None:
            operands.append(bass2jax.partition_id_tensor())
        outs = bass2jax._bass_exec_p.bind(
            *operands,
            out_avals=tuple(out_avals),
            in_names=tuple(all_in_names),
            out_names=tuple(out_names),
            lowering_input_output_aliases=(),
            sim_require_finite=True,
            sim_require_nnan=True,
            nc=nc,
        )
        return tuple(outs)

    devices = jax.devices()[:n_cores]
    mesh = Mesh(np.asarray(devices), ("core",))
    n_outs = len(out_names)
    in_specs = (PartitionSpec("core"),) * (n_params + n_outs)
    out_specs = (PartitionSpec("core"),) * n_outs
    donate = tuple(range(n_params, n_params + n_outs))
    fn = jax.jit(
        shard_map(_body, mesh=mesh, in_specs=in_specs, out_specs=out_specs,
                  check_rep=False),
        donate_argnums=donate, keep_unused=True)
    state = {"donate": None}

    def run(global_in_map):
        import jax as _jax
        args = [global_in_map[n] for n in in_names]
        if state["donate"] is None:
            state["donate"] = [
                np.zeros((n_cores * a.shape[0], *a.shape[1:]), a.dtype)
                for a in out_avals]
        outs = fn(*args, *state["donate"])
        _jax.block_until_ready(outs)
        state["donate"] = list(outs)
        return {n: o for n, o in zip(out_names, outs)}

    return run


def kernel(in_features, q_proj_weight, k_proj_weight, v_proj_weight,
           o_proj_weight, token_positions, d_model=1024, num_heads=16,
           **_ignored):
    global last_exec_ns
    assert int(d_model) == D and int(num_heads) == H
    in_maps = make_in_maps(in_features, q_proj_weight, k_proj_weight,
                           v_proj_weight, o_proj_weight, token_positions)
    if "nc" not in _RUN_CACHE:
        _RUN_CACHE["nc"] = build_nc()
    nc = _RUN_CACHE["nc"]

    trace = bool(int(os.environ.get("KERNEL_TRACE", "0")))
    if _axon_active() and not trace:
        if "runner" not in _RUN_CACHE:
            _RUN_CACHE["runner"] = _make_cached_runner(nc)
        x = np.ascontiguousarray(np.asarray(in_features, np.float32))
        g = {"xs": x.reshape(NCORES * 512, D)}
        for nmm, w in zip(("wq", "wk", "wv", "wo"),
                          (q_proj_weight, k_proj_weight, v_proj_weight,
                           o_proj_weight)):
            w = np.asarray(w, np.float32)
            g[nmm] = np.concatenate([w, w], axis=0)
        cs, sn = _tables()
        g["cs"] = np.tile(cs, (NCORES, 1))
        g["sn"] = np.tile(sn, (NCORES, 1))
        outs = _RUN_CACHE["runner"](g)
        last_exec_ns = None
        parts = np.asarray(outs["out_part"]).reshape(NCORES, S, CW)
    else:
        res = bass_utils.run_bass_kernel_spmd(nc, in_maps,
                                              core_ids=list(range(NCORES)),
                                              trace=trace)
        last_exec_ns = res.exec_time_ns
        parts = np.stack([r["out_part"].astype(np.float32)
                          for r in res.results])

    out = np.empty((B, S, D), np.float32)
    for c in range(NCORES):
        b, gidx = c // 4, c % 4
        out[b, :, gidx * CW:(gidx + 1) * CW] = parts[c]
    return out
